# revision 1
# baseline (speedup 1.0000x reference)
"""Trainium2 Bass kernel for nn_ImprintedModel (retrieval_knn).

Computes y[c, b] = max over the 32 proxies p of class c of
    (w1[p] / ||w1[p]||) . (data[b] / ||data[b]||)
for data [4096, 512], w1 [64000, 512] (2000 classes x 32 proxies),
output [2000, 4096] fp32.

Sharding: w1 rows (and hence classes) split across 8 cores (8000 rows =
250 classes per core); data replicated. Each core computes its 250
output rows for all 4096 batch columns; host concatenates/transposes.

Host-side layout prep (no arithmetic beyond float32r grid rounding):
data is also shipped pre-transposed [512, 4096] so the contraction dim
sits on partitions without burning TensorE transposes on it.

Per-core device algorithm (fp32 storage, matmul in float32r):
  1. DMA dataT chunks straight to SBUF; compute 1/max(||data row||,eps)
     from the natural copy on ScalarE (Square + accum_out) + VectorE.
  2. Stream w in n-tile PAIRS of 2x512 rows: normalize rows on ScalarE,
     PE-transpose to wnT[j][k] [128e, 512p].  Prep for pair p+1 is
     emitted before pair p's matmuls so TensorE never waits on it.
  3. For each batch m-tile (128 rows): 8 accumulating float32r matmuls
     fill one [128b, 1024p] PSUM tile (two banks, two 512-row groups),
     then one VectorE tensor_reduce(max) over the 3D view
     [128, ncls, 32prox] -> out_sb[128, ncls] class scores.
  4. Final per-row scale by dnorm_inv, DMA out [4096, 250] per core.
"""

import numpy as np

# Problem shapes (hardcoded; harness always calls with these).
B = 4096
E = 512
C = 2000
PROXIES = 32
P = C * PROXIES
N_CORES = 8
P_SHARD = P // N_CORES      # 8000 w rows per core
C_SHARD = C // N_CORES      # 250 classes per core
EPS = 1e-12

PE_TILE = 128               # partitions / PE array edge
NW = 512                    # w rows per n-tile (16 classes)


def build_bass_kernel(b=B, e=E, p_shard=P_SHARD, proxies=PROXIES):
    from concourse import bacc, mybir, masks
    from concourse.tile import TileContext

    f32 = mybir.dt.float32
    f32r = mybir.dt.float32r
    AF = mybir.ActivationFunctionType
    AX = mybir.AxisListType
    OP = mybir.AluOpType

    assert e % PE_TILE == 0 and b % PE_TILE == 0 and p_shard % proxies == 0
    KC = e // PE_TILE               # contraction chunks (4)
    MT = b // PE_TILE               # batch m-tiles (32)
    c_shard = p_shard // proxies    # classes per core (250)

    # n-tiles over the w rows, then grouped into pairs sharing a PSUM tile
    n_tiles = []
    rs = 0
    while rs < p_shard:
        nw = min(NW, p_shard - rs)
        assert nw % proxies == 0
        n_tiles.append((rs, nw))
        rs += nw
    n_pairs = [tuple(n_tiles[i:i + 2]) for i in range(0, len(n_tiles), 2)]

    nc = bacc.Bacc("TRN2", target_bir_lowering=False, debug=False)
    data_d = nc.dram_tensor("data", [b, e], f32, kind="ExternalInput")
    dataT_d = nc.dram_tensor("dataT", [e, b], f32r, kind="ExternalInput")
    w_d = nc.dram_tensor("w", [p_shard, e], f32, kind="ExternalInput")
    out_d = nc.dram_tensor("out", [b, c_shard], f32, kind="ExternalOutput")

    with TileContext(nc) as tc:
        with tc.tile_pool(name="sbuf", bufs=1) as sb, \
             tc.tile_pool(name="mmps", bufs=3, space="PSUM") as psm, \
             tc.tile_pool(name="trps", bufs=2, space="PSUM") as pst:

            ident = sb.tile([PE_TILE, PE_TILE], f32, tag="ident")
            masks.make_identity(nc, ident[:])

            # dataT: one [128, B] tile per E-chunk, DMA'd directly (2MB each)
            dataT = [
                sb.tile([PE_TILE, b], f32r, tag=f"dT{k}", name=f"dT{k}")
                for k in range(KC)
            ]

            # 1/max(||data row||, eps); column m holds m-tile m's rows.
            dnorm = sb.tile([PE_TILE, MT], f32, tag="dnorm")

            def rownorm_recip(src, rows, dst_ap):
                """dst_ap [rows,1] = 1/max(||src row||, eps) on ACT+DVE."""
                sq = sb.tile([PE_TILE, e], f32, tag="sq", bufs=4, name="sq")
                ssq = sb.tile([PE_TILE, 1], f32, tag="ssq", bufs=8, name="ssq")
                nc.scalar.activation(sq[:rows], src, AF.Square,
                                     accum_out=ssq[:rows])
                nrm = sb.tile([PE_TILE, 1], f32, tag="nrm", bufs=8, name="nrm")
                nc.scalar.sqrt(nrm[:rows], ssq[:rows])
                nmx = sb.tile([PE_TILE, 1], f32, tag="nmx", bufs=8, name="nmx")
                nc.vector.tensor_scalar_max(nmx[:rows], nrm[:rows], EPS)
                nc.vector.reciprocal(dst_ap, nmx[:rows])

            # persistent per-m output accumulators [128, c_shard]
            out_sb = [
                sb.tile([PE_TILE, c_shard], f32, tag=f"osb{m}", name=f"osb{m}")
                for m in range(MT)
            ]

            def prep_pair(pair):
                """DMA + normalize + transpose one n-tile pair -> wnT."""
                wnT = [
                    [sb.tile([PE_TILE, NW], f32r, tag=f"wnT{j}_{k}", bufs=2,
                             name=f"wnT{j}_{k}") for k in range(KC)]
                    for j in range(len(pair))
                ]
                for j, (rs, nw) in enumerate(pair):
                    r = 0
                    while r < nw:
                        rows = min(PE_TILE, nw - r)
                        wnat = sb.tile([PE_TILE, e], f32, tag="wnat", bufs=8,
                                       name="wnat")
                        nc.sync.dma_start(wnat[:rows],
                                          w_d[rs + r: rs + r + rows, :])
                        rinv = sb.tile([PE_TILE, 1], f32, tag="rinv", bufs=8,
                                       name="rinv")
                        rownorm_recip(wnat[:rows], rows, rinv[:rows])
                        wn = sb.tile([PE_TILE, e], f32, tag="wn", bufs=8,
                                     name="wn")
                        nc.scalar.mul(wn[:rows], wnat[:rows], rinv[:rows])
                        for k in range(KC):
                            trp = pst.tile([PE_TILE, PE_TILE], f32,
                                           tag="trp", name="trp")
                            nc.tensor.transpose(
                                trp[:, :rows],
                                wn[:rows, k * 128:(k + 1) * 128],
                                ident[:rows, :rows])
                            # f32 -> f32r rounding happens in this copy
                            nc.scalar.copy(wnT[j][k][:, r:r + rows],
                                           trp[:, :rows])
                        r += rows
                return wnT

            def mm_pair(pair, wnT, c0):
                pw = sum(nw for (_, nw) in pair)        # 1024 or 832
                ncls = pw // proxies
                for m in range(MT):
                    ps = psm.tile([PE_TILE, 2 * NW], f32, tag="mmps",
                                  name="mmps")
                    off = 0
                    for j, (rs, nw) in enumerate(pair):
                        for k in range(KC):
                            nc.tensor.matmul(
                                ps[:, off:off + nw],
                                dataT[k][:, m * 128:(m + 1) * 128],
                                wnT[j][k][:, :nw],
                                start=(k == 0),
                                stop=(k == KC - 1),
                            )
                        off += nw
                    nc.vector.tensor_reduce(
                        out_sb[m][:, c0:c0 + ncls],
                        ps[:, :pw].rearrange("p (c g) -> p c g", g=proxies),
                        axis=AX.X,
                        op=OP.max,
                    )
                return ncls

            # ---- main loop, software-pipelined: prep pair p+1 sits
            # before pair p's matmuls in every engine's program order.
            # DMA emission order: pair-0 w tiles first, then dataT bulk,
            # then the rest — so the startup critical path is short.
            c0 = 0
            wnT_cur = prep_pair(n_pairs[0])
            for k in range(KC):
                nc.sync.dma_start(dataT[k][:],
                                  dataT_d[k * 128:(k + 1) * 128, :])
            for pi, pair in enumerate(n_pairs):
                wnT_next = (prep_pair(n_pairs[pi + 1])
                            if pi + 1 < len(n_pairs) else None)
                c0 += mm_pair(pair, wnT_cur, c0)
                wnT_cur = wnT_next
                if pi == 0:
                    # data norms (needed only by the epilogue): emitted here
                    # so their DMA/ACT work stays off the startup path
                    for m in range(MT):
                        dnat = sb.tile([PE_TILE, e], f32, tag="dnat", bufs=4,
                                       name="dnat")
                        nc.sync.dma_start(dnat[:],
                                          data_d[m * 128:(m + 1) * 128, :])
                        rownorm_recip(dnat[:], PE_TILE, dnorm[:, m:m + 1])

            # ---- epilogue: scale by data-norm reciprocal, store ----
            for m in range(MT):
                fin = sb.tile([PE_TILE, c_shard], f32, tag="fin", bufs=4,
                              name="fin")
                nc.scalar.mul(fin[:], out_sb[m][:], dnorm[:, m:m + 1])
                nc.sync.dma_start(out_d[m * 128:(m + 1) * 128, :], fin[:])

    nc.compile()
    return nc


_NC_CACHE = {}


def _get_nc(key, **kwargs):
    if key not in _NC_CACHE:
        _NC_CACHE[key] = build_bass_kernel(**kwargs)
    return _NC_CACHE[key]


def round_f32r(x):
    """Round fp32 to the float32r grid: bf16 hi + bf16 lo residual."""
    import ml_dtypes
    hi = x.astype(ml_dtypes.bfloat16).astype(np.float32)
    lo = (x - hi).astype(ml_dtypes.bfloat16).astype(np.float32)
    return hi + lo


def kernel(data, w1, segment_ids=None):
    """Full-input entry point: shards internally across 8 NeuronCores."""
    from concourse.bass_utils import run_bass_kernel_spmd

    data = np.ascontiguousarray(np.asarray(data), dtype=np.float32)
    w1 = np.ascontiguousarray(np.asarray(w1), dtype=np.float32)
    assert data.shape == (B, E) and w1.shape == (P, E)
    dataT = np.ascontiguousarray(round_f32r(data).T)

    nc = _get_nc("full")
    in_maps = [
        {"data": data, "dataT": dataT,
         "w": w1[i * P_SHARD:(i + 1) * P_SHARD]}
        for i in range(N_CORES)
    ]
    res = run_bass_kernel_spmd(nc, in_maps, core_ids=list(range(N_CORES)))
    out = np.empty((C, B), dtype=np.float32)
    for i in range(N_CORES):
        out[i * C_SHARD:(i + 1) * C_SHARD, :] = res.results[i]["out"].T
    return out

